# revision 19
# baseline (speedup 1.0000x reference)
# CPAMDec attention decoder kernel for Trainium2 (Bass/Tile), SPMD over 8 cores.
#
# Reference computation (per batch n):
#   q = (Wq @ x_n + bq)            # (C4, HW)   1x1 conv as matmul
#   k = y_n @ Wk.T + bk            # (K, C4)
#   v = y_n @ Wv.T + bv            # (K, C)
#   energy[p,kk] = sum_m q[m,p] k[kk,m]          # (HW, K)
#   att = softmax(energy, axis=-1)
#   out_attn[c,p] = sum_kk v[kk,c] att[p,kk]     # (C, HW)
#   out = scale * out_attn + x
#
# Sharding: data-parallel over N across the 8 cores (1 batch each); weights
# replicated.  Host-side prep packs every weight into the exact per-partition
# SBUF byte layout (so each DMA line is 2-8KB contiguous) and folds `scale`
# into Wv/bv.  bv is applied via a rank-1 PSUM accumulate onto v
# (ones(1,K).T @ sbv(1,C)), exact because softmax rows sum to 1.
#
# The attention block is computed entirely in the transposed (K, positions)
# layout so no on-chip transposes are needed:
#   eT = kT.T @ q                (PE, (K, 512) per chunk)
#   denom = ones(K,1).T @ exp(eT)    (PE reduces over K partitions)
#   bcast = ones(1,K).T @ recip      (PE broadcasts 1/denom over K rows)
#   attT = exp(eT) * bcast           (DVE, one multiply)
#   U    = v.T @ attT                (PE)
# Softmax skips max-subtraction: |energy| <~ 25, comfortably inside fp32 exp
# range, and softmax is shift-invariant so the result is identical.
#
# x streams in DMA chunks of (C, 1024) (4KB lines); all loads share the sync
# HWDGE ring in need-order (weights first, so they are never starved behind
# x packets at the SDMA round-robin); output stores ride the scalar ring.
#
# All matmuls use float32r (1 cycle/row vs fp32's 4; the PE sits at ~60%
# occupancy in this DMA-bound kernel, so HAM keeps it at 1.2 GHz — cycle
# count is what matters).  The residual path (x, +) stays exact fp32.  The
# BIR verifier requires fp32r matmul inputs to be *produced* as float32r,
# hence the bitcast views on the producing instructions (bytes unchanged for
# DMA; ACT/DVE round on write).

import numpy as np

import concourse.bacc as bacc
import concourse.mybir as mybir
import concourse.tile as tile
from concourse.bass import ts
from concourse.bass_utils import run_bass_kernel_spmd

N, C, H, W = 8, 512, 64, 64
HW = H * W          # 4096
K = 64              # gathering centers
C4 = C // 4         # 128
A = C // 128        # 4 c-chunks of 128
DCHUNK = 1024       # hw positions per DMA chunk
ND = HW // DCHUNK   # 4
CHUNK = 512         # hw positions per compute chunk
NJ = HW // CHUNK    # 8
JPD = DCHUNK // CHUNK  # compute chunks per DMA chunk

F32 = mybir.dt.float32
F32R = mybir.dt.float32r

Ident = mybir.ActivationFunctionType.Identity
Exp = mybir.ActivationFunctionType.Exp
Copy = mybir.ActivationFunctionType.Copy
Ln = mybir.ActivationFunctionType.Ln


def build(f32r: bool = True):
    nc = bacc.Bacc("TRN2", target_bir_lowering=False, debug=False)

    x_d = nc.dram_tensor("x", [C, HW], F32, kind="ExternalInput").ap()
    # packed layouts: element [p, a*m + j] = T[a*128 + p, j] for T with
    # 128-chunked rows (see prep_inputs)
    yt_d = nc.dram_tensor("yt", [128, A * K], F32, kind="ExternalInput").ap()
    wqt_d = nc.dram_tensor("wqt", [128, A * C4], F32, kind="ExternalInput").ap()
    wkt_d = nc.dram_tensor("wkt", [128, A * C4], F32, kind="ExternalInput").ap()
    wvt_d = nc.dram_tensor("wvt", [128, A * C], F32, kind="ExternalInput").ap()
    bq_d = nc.dram_tensor("bq", [C4], F32, kind="ExternalInput").ap()
    bk_d = nc.dram_tensor("bk", [C4], F32, kind="ExternalInput").ap()
    sbv_d = nc.dram_tensor("sbv", [C], F32, kind="ExternalInput").ap()
    out_d = nc.dram_tensor("out", [C, HW], F32, kind="ExternalOutput").ap()

    def r(ap):
        # float32r view for fp32r-matmul operands and their producers.
        return ap.bitcast(F32R) if f32r else ap

    with tile.TileContext(nc) as tc:
        with (
            tc.tile_pool(name="const", bufs=1) as cp,
            tc.tile_pool(name="xin", bufs=3) as xp,
            tc.tile_pool(name="q", bufs=3) as qp,
            tc.tile_pool(name="soft", bufs=3) as sp,
            tc.tile_pool(name="attT", bufs=3) as ap_,
            tc.tile_pool(name="osb", bufs=2) as op_,
            tc.tile_pool(name="ps_q", bufs=2, space="PSUM") as ps_q,
            tc.tile_pool(name="ps_et", bufs=2, space="PSUM") as ps_et,
            tc.tile_pool(name="ps_d", bufs=1, space="PSUM") as ps_d,
            tc.tile_pool(name="ps_o", bufs=3, space="PSUM") as ps_o,
        ):
            # ---- all loads on the sync ring, strictly in need-order
            bq_t = cp.tile([C4, 1], F32)
            nc.sync.dma_start(bq_t[:], bq_d.unsqueeze(1))
            wq = cp.tile([128, A * C4], F32)
            nc.sync.dma_start(r(wq[:]), r(wqt_d))

            xts = []
            def load_x(d, split=False):
                xt = xp.tile([128, A, DCHUNK], F32, tag="xt", name=f"xt{d}")
                # halves land (and unblock consumers) independently
                nh = 2 if split else 1
                hw_ = DCHUNK // nh
                for h in range(nh):
                    nc.sync.dma_start(
                        r(xt[:, :, ts(h, hw_)]),
                        r(
                            x_d[:, d * DCHUNK + h * hw_ : d * DCHUNK + (h + 1) * hw_]
                            .rearrange("(a p) q -> p a q", p=128)
                        ),
                    )
                return xt
            xts.append(load_x(0, split=True))

            wk = cp.tile([128, A * C4], F32)
            nc.sync.dma_start(r(wk[:]), r(wkt_d))
            yt = cp.tile([128, A * K], F32)
            nc.sync.dma_start(r(yt[:]), r(yt_d))
            bk_t = cp.tile([C4, 1], F32)
            nc.sync.dma_start(bk_t[:], bk_d.unsqueeze(1))
            sbv_row = cp.tile([1, C], F32)
            nc.sync.dma_start(sbv_row[:], sbv_d.unsqueeze(0))
            wv = cp.tile([128, A * C], F32)
            nc.sync.dma_start(r(wv[:]), r(wvt_d))

            # pre-trigger the Exp ACT table load so it overlaps the DMAs
            warm = cp.tile([1, 1], F32)
            nc.scalar.activation(warm[:], bq_t[0:1, :], Exp)

            # all-ones helpers, produced as f32r via ACT copies (memset
            # cannot write float32r directly)
            ones_raw = cp.tile([K, K], F32)
            nc.gpsimd.memset(ones_raw[:], 1.0)
            ones_sq = cp.tile([K, K], F32)    # f32r, lhsT for the K-reduce
            nc.scalar.activation(r(ones_sq[:]), ones_raw[:], Copy)
            ones_row = cp.tile([1, K], F32)   # fp32, for the rank-1 v bias
            nc.gpsimd.memset(ones_row[:], 1.0)

            # ---------- chunk-0 q first so PE starts as soon as wq+x0 land
            def q_proj(j, xt, xoff):
                psq = ps_q.tile([C4, CHUNK], F32, tag="psq", name=f"psq{j}")
                for a in range(A):
                    nc.tensor.matmul(
                        psq[:], r(wq[:, ts(a, C4)]), r(xt[:, a, ts(xoff, CHUNK)]),
                        start=(a == 0), stop=(a == A - 1),
                    )
                q_sb = qp.tile([C4, CHUNK], F32, tag="q", name=f"q{j}")
                nc.scalar.activation(r(q_sb[:]), psq[:], Ident, bias=bq_t[:])
                return q_sb

            q0 = q_proj(0, xts[0], 0)

            # ---------- k^T = WkT.T @ yT + bk : (C4, K) ----------
            ps_k = ps_q.tile([C4, K], F32, tag="psq")
            for a in range(A):
                nc.tensor.matmul(
                    ps_k[:], r(wk[:, ts(a, C4)]), r(yt[:, ts(a, K)]),
                    start=(a == 0), stop=(a == A - 1),
                )
            kT = cp.tile([C4, K], F32)
            nc.scalar.activation(r(kT[:]), ps_k[:], Ident, bias=bk_t[:])

            v_sb = cp.tile([K, C], F32)

            def v_setup():
                # v_s = yT.T @ WvTs + ones.T @ sbv : (K, C); emitted after
                # chunk-0 softmax so the DVE isn't head-blocked on wv/sbv.
                ps_v = ps_et.tile([K, C], F32, tag="pset")
                for a in range(A):
                    nc.tensor.matmul(
                        ps_v[:], r(yt[:, ts(a, K)]), r(wv[:, ts(a, C)]),
                        start=(a == 0), stop=False,
                    )
                nc.tensor.matmul(
                    ps_v[:], ones_row[:], sbv_row[:], start=False, stop=True
                )
                nc.vector.tensor_copy(r(v_sb[:]), ps_v[:])

            # ---------- streaming pipeline, software-pipelined with a
            # one-chunk skew: stage2(j-1) is emitted after stage1(j) so the
            # PE has stage-1 matmuls in its stream while the DVE computes
            # the reciprocal stage2 needs (otherwise the in-order PE stalls
            # ~3.4us at the bcast matmul every chunk).
            outts = {}
            state = {}

            def stage1(j):
                d, xoff = divmod(j, JPD)
                if d + 1 < ND and len(xts) == d + 1:
                    xts.append(load_x(d + 1))   # prefetch next DMA chunk
                xt = xts[d]

                q_sb = q0 if j == 0 else q_proj(j, xt, xoff)

                # eT = kT.T @ q : (K, CHUNK), one matmul
                pset = ps_et.tile([K, CHUNK], F32, tag="pset", name=f"pset{j}")
                nc.tensor.matmul(pset[:], r(kT[:]), r(q_sb[:]), start=True, stop=True)

                # exp, then a ones(K,K) matmul reduces over the K
                # partitions AND broadcasts the row-sum back to all K rows
                expT = sp.tile([K, CHUNK], F32, tag="expT", name=f"expT{j}")
                nc.scalar.activation(r(expT[:]), pset[:], Exp)
                psd = ps_d.tile([K, CHUNK], F32, tag="psd", name=f"psd{j}")
                nc.tensor.matmul(psd[:], r(ones_sq[:]), r(expT[:]),
                                 start=True, stop=True)
                # 1/denom via exp(-ln(denom)) on ACT: the DVE reciprocal is
                # per-lane iterative (~8 cyc/elem, ~3.3us for 512/lane); ACT
                # streams it at 1 elem/cyc and Ln/Exp share one table set.
                lnd = sp.tile([K, CHUNK], F32, tag="lnd", name=f"lnd{j}")
                nc.scalar.activation(lnd[:], psd[:], Ln)
                recip = sp.tile([K, CHUNK], F32, tag="recip", name=f"recip{j}")
                nc.scalar.activation(r(recip[:]), lnd[:], Exp, scale=-1.0)
                state[j] = (xt, xoff, d, expT, recip)

            def stage2(j):
                xt, xoff, d, expT, recip = state.pop(j)
                attT = ap_.tile([K, CHUNK], F32, tag="attT", name=f"attT{j}")
                nc.vector.tensor_mul(r(attT[:]), expT[:], recip[:])

                # out = v_s.T @ att^T + x : (C, CHUNK)
                if xoff == 0:
                    outts[d] = op_.tile([128, A, DCHUNK], F32, tag="outt",
                                        name=f"outt{d}")
                outt = outts[d]
                for a in range(A):
                    pso = ps_o.tile([128, CHUNK], F32, tag="pso", name=f"pso{j}_{a}")
                    nc.tensor.matmul(
                        pso[:], r(v_sb[:, ts(a, 128)]), r(attT[:]),
                        start=True, stop=True,
                    )
                    nc.vector.tensor_add(
                        outt[:, a, ts(xoff, CHUNK)], pso[:], xt[:, a, ts(xoff, CHUNK)]
                    )
                    if xoff == JPD - 1 and a % 2 == 1:
                        # (128, 2, 1024) = 1MB store, 4KB per-partition
                        # lines, via SWDGE on the otherwise-idle GpSimd so
                        # store triggers never block ACT or the load ring
                        nc.gpsimd.dma_start(
                            out_d[ts(a // 2, 256), ts(d, DCHUNK)]
                            .rearrange("(a p) q -> p a q", p=128),
                            outt[:, a - 1 : a + 1, :],
                        )

            stage1(0)
            stage1(1)
            v_setup()
            for j in range(2, NJ):
                stage2(j - 2)
                stage1(j)
            stage2(NJ - 2)
            stage2(NJ - 1)

    nc.compile()
    return nc


def _pack_rows(t, m):
    # (A*128, m) -> (128, A*m): out[p, a*m+j] = t[a*128+p, j]
    return np.ascontiguousarray(
        t.reshape(A, 128, m).transpose(1, 0, 2).reshape(128, A * m)
    )


def prep_inputs(x, y, Wq, bq, Wk, bk, Wv, bv, scale):
    """Host-side prep: per-core input maps (weights packed, scale folded)."""
    x = np.asarray(x, dtype=np.float32)
    y = np.asarray(y, dtype=np.float32)
    s = float(np.asarray(scale).reshape(-1)[0])
    shared = {
        "wqt": _pack_rows(np.asarray(Wq, np.float32).T, C4),
        "wkt": _pack_rows(np.asarray(Wk, np.float32).T, C4),
        "wvt": _pack_rows(np.asarray(Wv, np.float32).T * s, C),
        "bq": np.ascontiguousarray(np.asarray(bq, np.float32)),
        "bk": np.ascontiguousarray(np.asarray(bk, np.float32)),
        "sbv": np.ascontiguousarray(np.asarray(bv, np.float32) * s),
    }
    in_maps = []
    for n in range(N):
        in_maps.append(
            {
                "x": np.ascontiguousarray(x[n].reshape(C, HW)),
                "yt": _pack_rows(np.ascontiguousarray(y[n].T), K),
                **shared,
            }
        )
    return in_maps


_NC_CACHE = {}


def get_nc(f32r: bool = True):
    if f32r not in _NC_CACHE:
        _NC_CACHE[f32r] = build(f32r)
    return _NC_CACHE[f32r]


def kernel(x, y, Wq, bq, Wk, bk, Wv, bv, scale, **run_kwargs):
    nc = get_nc()
    in_maps = prep_inputs(x, y, Wq, bq, Wk, bk, Wv, bv, scale)
    res = run_bass_kernel_spmd(nc, in_maps, core_ids=list(range(N)), **run_kwargs)
    out = np.stack([res.results[n]["out"] for n in range(N)], axis=0)
    return out.reshape(N, C, H, W).astype(np.float32)


# revision 20
# speedup vs baseline: 1.0082x; 1.0082x over previous
# CPAMDec attention decoder kernel for Trainium2 (Bass/Tile), SPMD over 8 cores.
#
# Reference computation (per batch n):
#   q = (Wq @ x_n + bq)            # (C4, HW)   1x1 conv as matmul
#   k = y_n @ Wk.T + bk            # (K, C4)
#   v = y_n @ Wv.T + bv            # (K, C)
#   energy[p,kk] = sum_m q[m,p] k[kk,m]          # (HW, K)
#   att = softmax(energy, axis=-1)
#   out_attn[c,p] = sum_kk v[kk,c] att[p,kk]     # (C, HW)
#   out = scale * out_attn + x
#
# Sharding: data-parallel over N across the 8 cores (1 batch each); weights
# replicated.  Host-side prep packs every weight into the exact per-partition
# SBUF byte layout (so each DMA line is 2-8KB contiguous) and folds `scale`
# into Wv/bv.  bv is applied via a rank-1 PSUM accumulate onto v
# (ones(1,128).T @ sbv(1,C)), exact because softmax rows sum to 1.
#
# The attention block runs in the transposed (K, positions) layout, with all
# K-sized intermediates DUPLICATED to 128 partitions (ACT/DVE time only
# depends on the free dim, so the duplicate rows are free, and they let the
# out-bmm run as two concurrent row-tiled K=64 matmuls in the PE array):
#   eT2   = [kT|kT].T @ q               (PE, (128, 512): two stacked copies)
#   denom = ones(64,128).T @ exp(eT2)[0:64]   (PE: K-reduce + row-broadcast)
#   recip = exp(-ln(denom))             (ACT; DVE reciprocal is ~5x slower)
#   attT2 = exp(eT2) * recip            (DVE, one multiply)
#   U     = v2.T @ attT2                (PE, row-tiled pairs: rows 0:64 and
#                                        64:128 compute different c-tiles
#                                        concurrently)
# Softmax skips max-subtraction: |energy| <~ 25, comfortably inside fp32 exp
# range, and softmax is shift-invariant so the result is identical.
#
# x streams in DMA chunks of (C, 1024) (4KB lines) on the sync HWDGE ring in
# need-order (weights first); output stores go through SWDGE on the
# otherwise-idle GpSimd so store triggers never block loads or ACT compute.
# Stage2 of each chunk is emitted two chunks behind stage1 so the in-order
# PE stream always has independent matmuls while ACT/DVE finish a softmax.
#
# All matmuls use float32r (1 cycle/row vs fp32's 4; the PE sits at ~60%
# occupancy so HAM keeps it at 1.2 GHz — cycle count is what matters).  The
# residual path (x, +) stays exact fp32.  The BIR verifier requires fp32r
# matmul inputs to be *produced* as float32r, hence the bitcast views on the
# producing instructions (bytes unchanged for DMA; ACT/DVE round on write).

import numpy as np

import concourse.bacc as bacc
import concourse.mybir as mybir
import concourse.tile as tile
from concourse.bass import ts
from concourse.bass_utils import run_bass_kernel_spmd

N, C, H, W = 8, 512, 64, 64
HW = H * W          # 4096
K = 64              # gathering centers
C4 = C // 4         # 128
A = C // 128        # 4 c-chunks of 128
DCHUNK = 1024       # hw positions per DMA chunk
ND = HW // DCHUNK   # 4
CHUNK = 512         # hw positions per compute chunk
NJ = HW // CHUNK    # 8
JPD = DCHUNK // CHUNK  # compute chunks per DMA chunk

F32 = mybir.dt.float32
F32R = mybir.dt.float32r

Ident = mybir.ActivationFunctionType.Identity
Exp = mybir.ActivationFunctionType.Exp
Copy = mybir.ActivationFunctionType.Copy
Ln = mybir.ActivationFunctionType.Ln


def build(f32r: bool = True):
    nc = bacc.Bacc("TRN2", target_bir_lowering=False, debug=False)

    x_d = nc.dram_tensor("x", [C, HW], F32, kind="ExternalInput").ap()
    # packed layouts: element [p, a*m + j] = T[a*128 + p, j] for T with
    # 128-chunked rows; yt is additionally duplicated along its free dim
    # (see prep_inputs)
    yt_d = nc.dram_tensor("yt", [128, A * 2 * K], F32, kind="ExternalInput").ap()
    wqt_d = nc.dram_tensor("wqt", [128, A * C4], F32, kind="ExternalInput").ap()
    wkt_d = nc.dram_tensor("wkt", [128, A * C4], F32, kind="ExternalInput").ap()
    wvt_d = nc.dram_tensor("wvt", [128, A * C], F32, kind="ExternalInput").ap()
    bq_d = nc.dram_tensor("bq", [C4], F32, kind="ExternalInput").ap()
    bk_d = nc.dram_tensor("bk", [C4], F32, kind="ExternalInput").ap()
    sbv_d = nc.dram_tensor("sbv", [C], F32, kind="ExternalInput").ap()
    out_d = nc.dram_tensor("out", [C, HW], F32, kind="ExternalOutput").ap()

    K2 = 2 * K

    def r(ap):
        # float32r view for fp32r-matmul operands and their producers.
        return ap.bitcast(F32R) if f32r else ap

    with tile.TileContext(nc) as tc:
        with (
            tc.tile_pool(name="const", bufs=1) as cp,
            tc.tile_pool(name="xin", bufs=3) as xp,
            tc.tile_pool(name="q", bufs=3) as qp,
            tc.tile_pool(name="soft", bufs=3) as sp,
            tc.tile_pool(name="attT", bufs=3) as ap_,
            tc.tile_pool(name="osb", bufs=2) as op_,
            tc.tile_pool(name="ps_q", bufs=2, space="PSUM") as ps_q,
            tc.tile_pool(name="ps_et", bufs=2, space="PSUM") as ps_et,
            tc.tile_pool(name="ps_d", bufs=1, space="PSUM") as ps_d,
            tc.tile_pool(name="ps_o", bufs=3, space="PSUM") as ps_o,
        ):
            # ---- all loads on the sync ring, strictly in need-order
            bq_t = cp.tile([C4, 1], F32)
            nc.sync.dma_start(bq_t[:], bq_d.unsqueeze(1))
            wq = cp.tile([128, A * C4], F32)
            nc.sync.dma_start(r(wq[:]), r(wqt_d))

            xts = []
            def load_x(d, split=False):
                xt = xp.tile([128, A, DCHUNK], F32, tag="xt", name=f"xt{d}")
                # halves land (and unblock consumers) independently
                nh = 2 if split else 1
                hw_ = DCHUNK // nh
                for h in range(nh):
                    nc.sync.dma_start(
                        r(xt[:, :, ts(h, hw_)]),
                        r(
                            x_d[:, d * DCHUNK + h * hw_ : d * DCHUNK + (h + 1) * hw_]
                            .rearrange("(a p) q -> p a q", p=128)
                        ),
                    )
                return xt
            xts.append(load_x(0, split=True))

            wk = cp.tile([128, A * C4], F32)
            nc.sync.dma_start(r(wk[:]), r(wkt_d))
            yt = cp.tile([128, A * K2], F32)
            nc.sync.dma_start(r(yt[:]), r(yt_d))
            bk_t = cp.tile([C4, 1], F32)
            nc.sync.dma_start(bk_t[:], bk_d.unsqueeze(1))
            sbv_row = cp.tile([1, C], F32)
            nc.sync.dma_start(sbv_row[:], sbv_d.unsqueeze(0))
            wv = cp.tile([128, A * C], F32)
            nc.sync.dma_start(r(wv[:]), r(wvt_d))

            # pre-trigger the Exp ACT table load so it overlaps the DMAs
            warm = cp.tile([1, 1], F32)
            nc.scalar.activation(warm[:], bq_t[0:1, :], Exp)

            # f32r all-ones (memset cannot write float32r) and f32r sbv
            ones_raw = cp.tile([K, C4], F32)
            nc.gpsimd.memset(ones_raw[:], 1.0)
            ones_f = cp.tile([K, C4], F32)
            nc.scalar.activation(r(ones_f[:]), ones_raw[:], Copy)
            sbv_f = cp.tile([1, C], F32)
            nc.scalar.activation(r(sbv_f[:]), sbv_row[:], Copy)

            # ---------- chunk-0 q first so PE starts as soon as wq+x0 land
            def q_proj(j, xt, xoff):
                psq = ps_q.tile([C4, CHUNK], F32, tag="psq", name=f"psq{j}")
                for a in range(A):
                    nc.tensor.matmul(
                        psq[:], r(wq[:, ts(a, C4)]), r(xt[:, a, ts(xoff, CHUNK)]),
                        start=(a == 0), stop=(a == A - 1),
                    )
                q_sb = qp.tile([C4, CHUNK], F32, tag="q", name=f"q{j}")
                nc.scalar.activation(r(q_sb[:]), psq[:], Ident, bias=bq_t[:])
                return q_sb

            q0 = q_proj(0, xts[0], 0)

            # ---------- kT2 = [kT|kT] : (C4, 2K) ----------
            ps_k = ps_q.tile([C4, K], F32, tag="psq")
            for a in range(A):
                nc.tensor.matmul(
                    ps_k[:], r(wk[:, ts(a, C4)]), r(yt[:, a * K2 : a * K2 + K]),
                    start=(a == 0), stop=(a == A - 1),
                )
            kT2 = cp.tile([C4, K2], F32)
            nc.scalar.activation(r(kT2[:, 0:K]), ps_k[:], Ident, bias=bk_t[:])
            nc.scalar.activation(r(kT2[:, K:K2]), ps_k[:], Ident, bias=bk_t[:])

            v2_sb = cp.tile([128, C], F32)

            def v_setup():
                # v2 = [v; v] stacked on 128 partitions: yt is host-duplicated
                # along free so each matmul emits M=128 rows (two v copies).
                ps_v = ps_et.tile([128, C], F32, tag="pset")
                for a in range(A):
                    nc.tensor.matmul(
                        ps_v[:], r(yt[:, ts(a, K2)]), r(wv[:, ts(a, C)]),
                        start=(a == 0), stop=False,
                    )
                nc.tensor.matmul(
                    ps_v[:], r(ones_f[0:1, :]), r(sbv_f[:]), start=False, stop=True
                )
                nc.vector.tensor_copy(r(v2_sb[:]), ps_v[:])

            # ---------- streaming pipeline, software-pipelined with a
            # two-chunk skew so the in-order PE stream never waits on the
            # ACT/DVE softmax of the chunk it is about to consume.
            outts = {}
            state = {}

            def stage1(j):
                d, xoff = divmod(j, JPD)
                if d + 1 < ND and len(xts) == d + 1:
                    xts.append(load_x(d + 1))   # prefetch next DMA chunk
                xt = xts[d]

                q_sb = q0 if j == 0 else q_proj(j, xt, xoff)

                # eT2 = kT2.T @ q : (128, CHUNK), rows 64:128 duplicate 0:64
                pset = ps_et.tile([128, CHUNK], F32, tag="pset", name=f"pset{j}")
                nc.tensor.matmul(pset[:], r(kT2[:]), r(q_sb[:]), start=True, stop=True)

                expT = sp.tile([128, CHUNK], F32, tag="expT", name=f"expT{j}")
                nc.scalar.activation(r(expT[:]), pset[:], Exp)
                # ones(64,128) matmul: reduces exp over the K partitions and
                # broadcasts the row-sum to all 128 rows in one shot
                psd = ps_d.tile([128, CHUNK], F32, tag="psd", name=f"psd{j}")
                nc.tensor.matmul(psd[:], r(ones_f[:]), r(expT[0:K, :]),
                                 start=True, stop=True)
                # 1/denom via exp(-ln(denom)) on ACT: the DVE reciprocal is
                # per-lane iterative (~8 cyc/elem, ~3.3us for 512/lane); ACT
                # streams it at 1 elem/cyc and Ln/Exp share one table set.
                lnd = sp.tile([128, CHUNK], F32, tag="lnd", name=f"lnd{j}")
                nc.scalar.activation(lnd[:], psd[:], Ln)
                recip = sp.tile([128, CHUNK], F32, tag="recip", name=f"recip{j}")
                nc.scalar.activation(r(recip[:]), lnd[:], Exp, scale=-1.0)
                state[j] = (xt, xoff, d, expT, recip)

            def stage2(j):
                xt, xoff, d, expT, recip = state.pop(j)
                attT = ap_.tile([128, CHUNK], F32, tag="attT", name=f"attT{j}")
                nc.vector.tensor_mul(r(attT[:]), expT[:], recip[:])

                # out = v2.T @ attT2 + x : (C, CHUNK); consecutive a use
                # disjoint PE row groups (0:64 / 64:128) -> concurrent MMs
                if xoff == 0:
                    outts[d] = op_.tile([128, A, DCHUNK], F32, tag="outt",
                                        name=f"outt{d}")
                outt = outts[d]
                for a in range(A):
                    rows = ts(a % 2, K)
                    pso = ps_o.tile([128, CHUNK], F32, tag="pso", name=f"pso{j}_{a}")
                    nc.tensor.matmul(
                        pso[:], r(v2_sb[rows, ts(a, 128)]), r(attT[rows, :]),
                        start=True, stop=True,
                    )
                    nc.vector.tensor_add(
                        outt[:, a, ts(xoff, CHUNK)], pso[:], xt[:, a, ts(xoff, CHUNK)]
                    )
                    if xoff == JPD - 1:
                        # (128, 1024) = 512KB store, 4KB per-partition lines,
                        # via SWDGE on the otherwise-idle GpSimd
                        nc.gpsimd.dma_start(
                            out_d[ts(a, 128), ts(d, DCHUNK)], outt[:, a, :]
                        )

            stage1(0)
            stage1(1)
            v_setup()
            for j in range(2, NJ):
                stage2(j - 2)
                stage1(j)
            stage2(NJ - 2)
            stage2(NJ - 1)

    nc.compile()
    return nc


def _pack_rows(t, m):
    # (A*128, m) -> (128, A*m): out[p, a*m+j] = t[a*128+p, j]
    return np.ascontiguousarray(
        t.reshape(A, 128, m).transpose(1, 0, 2).reshape(128, A * m)
    )


def prep_inputs(x, y, Wq, bq, Wk, bk, Wv, bv, scale):
    """Host-side prep: per-core input maps (weights packed, scale folded)."""
    x = np.asarray(x, dtype=np.float32)
    y = np.asarray(y, dtype=np.float32)
    s = float(np.asarray(scale).reshape(-1)[0])
    shared = {
        "wqt": _pack_rows(np.asarray(Wq, np.float32).T, C4),
        "wkt": _pack_rows(np.asarray(Wk, np.float32).T, C4),
        "wvt": _pack_rows(np.asarray(Wv, np.float32).T * s, C),
        "bq": np.ascontiguousarray(np.asarray(bq, np.float32)),
        "bk": np.ascontiguousarray(np.asarray(bk, np.float32)),
        "sbv": np.ascontiguousarray(np.asarray(bv, np.float32) * s),
    }
    in_maps = []
    for n in range(N):
        ytn = np.ascontiguousarray(y[n].T)              # (C, K)
        ytn2 = np.concatenate([ytn, ytn], axis=1)       # (C, 2K) duplicated
        in_maps.append(
            {
                "x": np.ascontiguousarray(x[n].reshape(C, HW)),
                "yt": _pack_rows(ytn2, 2 * K),
                **shared,
            }
        )
    return in_maps


_NC_CACHE = {}


def get_nc(f32r: bool = True):
    if f32r not in _NC_CACHE:
        _NC_CACHE[f32r] = build(f32r)
    return _NC_CACHE[f32r]


def kernel(x, y, Wq, bq, Wk, bk, Wv, bv, scale, **run_kwargs):
    nc = get_nc()
    in_maps = prep_inputs(x, y, Wq, bq, Wk, bk, Wv, bv, scale)
    res = run_bass_kernel_spmd(nc, in_maps, core_ids=list(range(N)), **run_kwargs)
    out = np.stack([res.results[n]["out"] for n in range(N)], axis=0)
    return out.reshape(N, C, H, W).astype(np.float32)
